# revision 19
# baseline (speedup 1.0000x reference)
"""Trainium2 Bass kernel for nn_CrossAttention (8-core data-parallel over batch).

Reference math (per batch b, chunk c):
  en = LayerNorm(e) ; q = en@Wq+bq ; k = h@Wk+bk ; v = h@Wv+bv
  attn = softmax(q@k^T / 8) ; o = attn@v ; out = o@Wo + bo + e

Host-side folding:  q = ((e-mu)*rstd) @ (ln_g[:,None]*Wq/8) + (ln_b@Wq+bq)/8
so the on-chip LN is just (e-mu)*rstd (one ACT op per tile).

On-chip layout strategy (all matmuls fp32r):
  - activations transposed to d-major ("T") layout via PE transposes
  - qT/kT/vT projections with weights as stationary lhsT
  - per-head-pair paired K=64 matmuls via tile_position (0,0)/(64,64)
  - softmax denominators via one accumulated E2 matmul -> psum [12, 256]
  - 1/den partition-broadcast via K=12 matmul with R_p selector constants
  - AV pair-matmuls write oT [do, r] layout directly; O-proj; PE transpose
    back to row-major with the residual fused into the psum->sbuf copy.
"""

import numpy as np

B, C, N, S, D = 8, 32, 4, 64, 768
NH, DK = 12, 64
R = N * S          # 256 rows per chunk
KO = D // 128      # 6 partition blocks of d
NP = 6             # head pairs
LN_EPS = 1e-5
GROUP = 4          # chunks per h/kv batch group

_prog_cache = {}


def _build(n_chunks):
    import concourse.bass as bass
    import concourse.tile as tile
    from concourse import bacc, mybir

    F32 = mybir.dt.float32
    F32R = mybir.dt.float32r
    AF = mybir.ActivationFunctionType
    ALU = mybir.AluOpType

    nc = bacc.Bacc()

    d_e = nc.dram_tensor("e", [n_chunks, R, D], F32, kind="ExternalInput")
    d_h = nc.dram_tensor("h", [n_chunks, S, D], F32R, kind="ExternalInput")
    d_wq = nc.dram_tensor("wq", [KO, 128, D], F32R, kind="ExternalInput")
    d_wk = nc.dram_tensor("wk", [KO, 128, D], F32R, kind="ExternalInput")
    d_wv = nc.dram_tensor("wv", [KO, 128, D], F32R, kind="ExternalInput")
    d_wo = nc.dram_tensor("wo", [KO, 128, D], F32R, kind="ExternalInput")
    d_bq = nc.dram_tensor("bq", [128, KO], F32, kind="ExternalInput")
    d_bk = nc.dram_tensor("bk", [128, KO], F32, kind="ExternalInput")
    d_bo = nc.dram_tensor("bo", [128, KO], F32, kind="ExternalInput")
    d_bvr = nc.dram_tensor("bvr", [128, D], F32, kind="ExternalInput")
    d_e2c = nc.dram_tensor("e2c", [128, NP, NH], F32R, kind="ExternalInput")
    d_rpc = nc.dram_tensor("rpc", [NH, NP, 128], F32R, kind="ExternalInput")
    d_id = nc.dram_tensor("ident", [128, 128], F32R, kind="ExternalInput")
    d_zf = nc.dram_tensor("zf", [128, 1], F32R, kind="ExternalInput")
    d_ones = nc.dram_tensor("ones", [1, 512], F32R, kind="ExternalInput")
    d_bqr = nc.dram_tensor("bqr", [1, D], F32R, kind="ExternalInput")
    d_bkr = nc.dram_tensor("bkr", [1, D], F32R, kind="ExternalInput")
    d_bor = nc.dram_tensor("bor", [1, D], F32R, kind="ExternalInput")
    d_bvrr = nc.dram_tensor("bvrr", [1, D], F32R, kind="ExternalInput")
    d_out = nc.dram_tensor("out", [n_chunks, R, D], F32, kind="ExternalOutput")

    from contextlib import ExitStack
    with ExitStack() as ctx:
        tc = ctx.enter_context(tile.TileContext(nc))
        consts = ctx.enter_context(tc.tile_pool(name="consts", bufs=1))
        e_pool = ctx.enter_context(tc.tile_pool(name="e_pool", bufs=2))
        x_pool = ctx.enter_context(tc.tile_pool(name="x_pool", bufs=2))
        xT_pool = ctx.enter_context(tc.tile_pool(name="xT_pool", bufs=2))
        q_pool = ctx.enter_context(tc.tile_pool(name="q_pool", bufs=2))
        exp_pool = ctx.enter_context(tc.tile_pool(name="exp_pool", bufs=2))
        oT_pool = ctx.enter_context(tc.tile_pool(name="oT_pool", bufs=1))
        fT_pool = ctx.enter_context(tc.tile_pool(name="fT_pool", bufs=1))
        vd_pool = ctx.enter_context(tc.tile_pool(name="vd_pool", bufs=2))
        grp_pool = ctx.enter_context(tc.tile_pool(name="grp_pool", bufs=1))
        kt_pool = ctx.enter_context(tc.tile_pool(name="kt_pool", bufs=1))
        v2_pool = ctx.enter_context(tc.tile_pool(name="v2_pool", bufs=2))
        st_pool = ctx.enter_context(tc.tile_pool(name="st_pool", bufs=2))
        ps_t = ctx.enter_context(tc.tile_pool(name="ps_t", bufs=2, space="PSUM"))
        ps_mm = ctx.enter_context(tc.tile_pool(name="ps_mm", bufs=2, space="PSUM"))
        ps_s = ctx.enter_context(tc.tile_pool(name="ps_s", bufs=1, space="PSUM"))
        ps_b = ctx.enter_context(tc.tile_pool(name="ps_b", bufs=1, space="PSUM"))
        ps_a = ctx.enter_context(tc.tile_pool(name="ps_a", bufs=1, space="PSUM"))
        ps_d = ctx.enter_context(tc.tile_pool(name="ps_d", bufs=1, space="PSUM"))
        if True:
            # ---- constants ----
            wq = consts.tile([128, KO, D], F32R)
            wk = consts.tile([128, KO, D], F32R)
            wv = consts.tile([128, KO, D], F32R)
            wo = consts.tile([128, KO, D], F32R)
            nc.sync.dma_start(wq[:], d_wq[:].rearrange("k p d -> p k d"))
            nc.sync.dma_start(wk[:], d_wk[:].rearrange("k p d -> p k d"))
            nc.sync.dma_start(wv[:], d_wv[:].rearrange("k p d -> p k d"))
            nc.sync.dma_start(wo[:], d_wo[:].rearrange("k p d -> p k d"))
            bq = consts.tile([128, KO], F32)
            bk = consts.tile([128, KO], F32)
            bo = consts.tile([128, KO], F32)
            bvr = consts.tile([128, D], F32)
            nc.sync.dma_start(bq[:], d_bq[:])
            nc.sync.dma_start(bk[:], d_bk[:])
            nc.sync.dma_start(bo[:], d_bo[:])
            nc.sync.dma_start(bvr[:], d_bvr[:])
            e2c = consts.tile([128, NP, NH], F32R)
            rpc = consts.tile([NH, NP, 128], F32R)
            ident = consts.tile([128, 128], F32R)
            zf = consts.tile([128, 1], F32R)
            nc.sync.dma_start(zf[:], d_zf[:])
            ones = consts.tile([1, 512], F32R)
            nc.sync.dma_start(ones[:], d_ones[:])
            bqr = consts.tile([1, D], F32R)
            bkr = consts.tile([1, D], F32R)
            bor = consts.tile([1, D], F32R)
            bvrr = consts.tile([1, D], F32R)
            nc.sync.dma_start(bqr[:], d_bqr[:])
            nc.sync.dma_start(bkr[:], d_bkr[:])
            nc.sync.dma_start(bor[:], d_bor[:])
            nc.sync.dma_start(bvrr[:], d_bvrr[:])
            nc.sync.dma_start(e2c[:], d_e2c[:])
            nc.sync.dma_start(rpc[:], d_rpc[:])
            nc.sync.dma_start(ident[:], d_id[:])
            eps_t = consts.tile([128, 1], F32)
            nc.vector.memset(eps_t[:], LN_EPS)

            n_groups = n_chunks // GROUP
            for g in range(n_groups):
                # ---- group phase: h load, transpose, K/V projections ----
                hT4 = grp_pool.tile([128, KO, GROUP * S], F32R, tag="hT4")
                for hh in range(2):
                    h2 = grp_pool.tile([S, 2, D], F32R, tag="h2")
                    nc.sync.dma_start(
                        h2[:], d_h[g * GROUP + 2 * hh:g * GROUP + 2 * hh + 2]
                        .rearrange("c j d -> j c d"))
                    for c2 in range(2):
                        cc = 2 * hh + c2
                        for ko2 in range(KO // 2):
                            pt2 = ps_t.tile([128, 2, 128], F32R, tag="t")
                            for t in range(2):
                                ko = 2 * ko2 + t
                                nc.tensor.transpose(
                                    pt2[:, t, 0:S],
                                    h2[:, c2, ko * 128:(ko + 1) * 128],
                                    ident[0:S, 0:S])
                            nc.vector.tensor_copy(
                                hT4[:, 2 * ko2:2 * ko2 + 2, cc * S:(cc + 1) * S],
                                pt2[:, :, 0:S])

                # kT in block-diagonal pair layout: for head pair p, chunk cc:
                # kbd[0:64, p, cc, 0:64]   = kT(head 2p)   [d, j]
                # kbd[64:128, p, cc, 64:128] = kT(head 2p+1) [d, j]
                kbd = kt_pool.tile([128, NP, GROUP, 128], F32R, tag="kbd")
                nc.gpsimd.tensor_copy(
                    kbd[:], zf[:, None, None, 0:1].to_broadcast(
                        [128, NP, GROUP, 128]))
                for mo in range(KO):
                    pk = ps_mm.tile([128, 512], F32, tag="mm")
                    for ko in range(KO):
                        nc.tensor.matmul(
                            pk[:, 0:GROUP * S],
                            wk[:, ko, mo * 128:(mo + 1) * 128],
                            hT4[:, ko, :],
                            start=(ko == 0), stop=False)
                    nc.tensor.matmul(
                        pk[:, 0:GROUP * S],
                        bkr[:, mo * 128:(mo + 1) * 128],
                        ones[:, 0:GROUP * S], start=False, stop=True)
                    pkv = pk[:, 0:GROUP * S].rearrange("p (c j) -> p c j", c=GROUP)
                    nc.vector.tensor_copy(kbd[0:64, mo, :, 0:S], pkv[0:64])
                    nc.vector.tensor_copy(kbd[64:128, mo, :, S:128], pkv[64:128])

                v2 = []
                for st in range(GROUP // 2):
                    v2t = v2_pool.tile([128, D], F32R, tag="v2")
                    for n0, ns in ((0, 512), (512, 256)):
                        pv = ps_mm.tile([128, 512], F32, tag="mm")
                        for ko in range(KO):
                            nc.tensor.matmul(
                                pv[:, 0:ns],
                                hT4[:, ko, st * 128:(st + 1) * 128],
                                wv[:, ko, n0:n0 + ns],
                                start=(ko == 0), stop=False)
                        nc.tensor.matmul(
                            pv[:, 0:ns], ones[:, 0:128],
                            bvrr[:, n0:n0 + ns], start=False, stop=True)
                        nc.vector.tensor_copy(v2t[:, n0:n0 + ns], pv[:, 0:ns])
                    v2.append(v2t)

                for cc in range(GROUP):
                    c = g * GROUP + cc
                    # ---- load e, LayerNorm stats + apply ----
                    e_sb = e_pool.tile([128, 2, D], F32, tag="e")
                    nc.sync.dma_start(
                        e_sb[:], d_e[c].rearrange("(t p) d -> p t d", p=128))

                    stats = st_pool.tile([128, 2, 3, 6], F32, tag="stats")
                    mv = st_pool.tile([128, 2, 2], F32, tag="mv")
                    rstd = st_pool.tile([128, 2], F32, tag="rstd")
                    x_sb = x_pool.tile([128, 2, D], F32R, tag="x")
                    for t in range(2):
                        esl = e_sb[:, t, :].rearrange("p (s f) -> p s f", s=3)
                        for sg in range(3):
                            nc.vector.bn_stats(stats[:, t, sg, :], esl[:, sg, :])
                        nc.vector.bn_aggr(mv[:, t, :], stats[:, t, :, :])
                    nc.scalar.activation(
                        rstd[:], mv[:, :, 1], AF.Sqrt, bias=eps_t[:], scale=1.0)
                    nc.vector.reciprocal(rstd[:], rstd[:])
                    for t in range(2):
                        nc.vector.tensor_scalar(
                            out=x_sb[:, t, :], in0=e_sb[:, t, :],
                            scalar1=mv[:, t, 0:1], scalar2=rstd[:, t:t + 1],
                            op0=ALU.subtract, op1=ALU.mult)

                    # ---- transpose x to d-major ----
                    xT = xT_pool.tile([128, KO, R], F32R, tag="xT")
                    for ko in range(KO):
                        pt2 = ps_t.tile([128, 2, 128], F32R, tag="t")
                        for t in range(2):
                            nc.tensor.transpose(
                                pt2[:, t, :], x_sb[:, t, ko * 128:(ko + 1) * 128],
                                ident[:])
                        nc.vector.tensor_copy(xT[:, ko, :], pt2[:])

                    # ---- Q projection ----
                    qT = q_pool.tile([128, KO, R], F32R, tag="qT")
                    for mo in range(KO):
                        pq = ps_mm.tile([128, 512], F32, tag="mm")
                        for ko in range(KO):
                            nc.tensor.matmul(
                                pq[:, 0:R], wq[:, ko, mo * 128:(mo + 1) * 128],
                                xT[:, ko, :], start=(ko == 0), stop=False)
                        nc.tensor.matmul(
                            pq[:, 0:R], bqr[:, mo * 128:(mo + 1) * 128],
                            ones[:, 0:R], start=False, stop=True)
                        nc.scalar.copy(qT[:, mo, :], pq[:, 0:R])

                    # ---- v in block-diagonal pair layout:
                    # vbd[0:64, p, 0:64]    = v[j, head 2p dims]
                    # vbd[64:128, p, 64:128] = v[j, head 2p+1 dims]
                    v2t = v2[cc // 2]
                    pa = 64 * (cc % 2)
                    vbd = vd_pool.tile([128, NP, 128], F32R, tag="vbd")
                    nc.gpsimd.tensor_copy(
                        vbd[:], zf[:, None, 0:1].to_broadcast([128, NP, 128]))
                    v2v = v2t[pa:pa + 64, :].rearrange(
                        "p (np two dk) -> p np two dk", np=NP, two=2)
                    nc.gpsimd.tensor_copy(vbd[0:64, :, 0:DK], v2v[:, :, 0, :])
                    nc.gpsimd.tensor_copy(vbd[64:128, :, DK:128], v2v[:, :, 1, :])

                    # ---- attention: scores -> exp -> den -> recip -> bcast ->
                    #      normalize -> AV (all per head-pair, paired matmuls)
                    expT = exp_pool.tile([128, NP, R], F32R, tag="expT")
                    pden = ps_d.tile([NH, R], F32, tag="den")
                    for p in range(NP):
                        pscr = ps_s.tile([128, R], F32, tag="s")
                        nc.tensor.matmul(
                            pscr[:], kbd[:, p, cc, :], qT[:, p, :],
                            start=True, stop=True)
                        nc.scalar.activation(
                            expT[:, p, :], pscr[:], AF.Exp, bias=0.0, scale=1.0)
                        nc.tensor.matmul(
                            pden[:], e2c[:, p, :], expT[:, p, :],
                            start=(p == 0), stop=(p == NP - 1),
                            skip_group_check=True)

                    recip = st_pool.tile([NH, R], F32R, tag="recip")
                    with nc.allow_low_precision(reason="fp32r softmax denom"):
                        nc.vector.reciprocal(recip[:], pden[:])

                    oT = oT_pool.tile([128, KO, R], F32R, tag="oT")
                    for p in range(NP):
                        pav = ps_a.tile([128, R], F32, tag="a")
                        nc.tensor.matmul(
                            pav[:], vbd[:, p, :], expT[:, p, :],
                            start=True, stop=True)
                        pbc = ps_b.tile([128, R], F32, tag="b")
                        nc.tensor.matmul(
                            pbc[:], rpc[:, p, :], recip[:], start=True, stop=True)
                        bc_sb = st_pool.tile([128, R], F32, tag="bcsb")
                        nc.scalar.copy(bc_sb[:], pbc[:])
                        nc.vector.tensor_tensor(
                            out=oT[:, p, :], in0=pav[:], in1=bc_sb[:], op=ALU.mult)

                    # ---- O projection ----
                    fT = fT_pool.tile([128, KO, R], F32R, tag="fT")
                    for mo in range(KO):
                        pf = ps_mm.tile([128, 512], F32, tag="mm")
                        for ko in range(KO):
                            nc.tensor.matmul(
                                pf[:, 0:R], wo[:, ko, mo * 128:(mo + 1) * 128],
                                oT[:, ko, :], start=(ko == 0), stop=False)
                        nc.tensor.matmul(
                            pf[:, 0:R], bor[:, mo * 128:(mo + 1) * 128],
                            ones[:, 0:R], start=False, stop=True)
                        nc.vector.tensor_copy(fT[:, mo, :], pf[:, 0:R])

                    # ---- transpose back + residual, store ----
                    for t in range(2):
                        for mo2 in range(KO // 2):
                            pt2 = ps_t.tile([128, 2, 128], F32R, tag="t")
                            for i in range(2):
                                mo = 2 * mo2 + i
                                nc.tensor.transpose(
                                    pt2[:, i, :], fT[:, mo, t * 128:(t + 1) * 128],
                                    ident[:])
                            nc.vector.tensor_tensor(
                                out=e_sb[:, t, mo2 * 256:(mo2 + 1) * 256],
                                in0=pt2[:].bitcast(F32),
                                in1=e_sb[:, t, mo2 * 256:(mo2 + 1) * 256],
                                op=ALU.add)
                    nc.sync.dma_start(
                        d_out[c].rearrange("(t p) d -> p t d", p=128), e_sb[:])

    nc.compile()
    return nc


def _prep_consts(Wq, bq, Wk, bk, Wv, bv, Wo, bo, ln_g, ln_b):
    scale = 1.0 / np.sqrt(DK)
    Wq_eff = (ln_g[:, None] * Wq) * scale
    bq_eff = (ln_b @ Wq + bq) * scale

    def wl(w):
        return np.ascontiguousarray(w.reshape(KO, 128, D), dtype=np.float32)

    def bcol(b):
        return np.ascontiguousarray(b.reshape(KO, 128).T, dtype=np.float32)

    e2c = np.zeros((128, NP, NH), dtype=np.float32)
    for p in range(NP):
        e2c[0:64, p, 2 * p] = 1.0
        e2c[64:128, p, 2 * p + 1] = 1.0
    rpc = np.zeros((NH, NP, 128), dtype=np.float32)
    for p in range(NP):
        rpc[2 * p, p, 0:64] = 1.0
        rpc[2 * p + 1, p, 64:128] = 1.0

    return {
        "wq": wl(Wq_eff), "wk": wl(Wk), "wv": wl(Wv), "wo": wl(Wo),
        "bq": bcol(bq_eff), "bk": bcol(bk), "bo": bcol(bo),
        "bvr": np.ascontiguousarray(
            np.broadcast_to(bv, (128, D)), dtype=np.float32),
        "e2c": e2c, "rpc": rpc, "zf": np.zeros((128, 1), dtype=np.float32),
        "ones": np.ones((1, 512), dtype=np.float32),
        "bqr": np.ascontiguousarray(bq_eff.reshape(1, D), dtype=np.float32),
        "bkr": np.ascontiguousarray(bk.reshape(1, D), dtype=np.float32),
        "bor": np.ascontiguousarray(bo.reshape(1, D), dtype=np.float32),
        "bvrr": np.ascontiguousarray(bv.reshape(1, D), dtype=np.float32),
        "ident": np.eye(128, dtype=np.float32),
    }


def kernel(e, h, Wq, bq, Wk, bk, Wv, bv, Wo, bo, ln_g, ln_b):
    from concourse.bass_utils import run_bass_kernel_spmd

    e = np.asarray(e, dtype=np.float32)
    h = np.asarray(h, dtype=np.float32)
    n_chunks = e.shape[1]

    if n_chunks not in _prog_cache:
        _prog_cache[n_chunks] = _build(n_chunks)
    nc = _prog_cache[n_chunks]

    consts = _prep_consts(
        np.asarray(Wq, np.float32), np.asarray(bq, np.float32),
        np.asarray(Wk, np.float32), np.asarray(bk, np.float32),
        np.asarray(Wv, np.float32), np.asarray(bv, np.float32),
        np.asarray(Wo, np.float32), np.asarray(bo, np.float32),
        np.asarray(ln_g, np.float32), np.asarray(ln_b, np.float32))

    in_maps = []
    for b in range(B):
        m = dict(consts)
        m["e"] = np.ascontiguousarray(e[b].reshape(n_chunks, R, D))
        m["h"] = np.ascontiguousarray(h[b])
        in_maps.append(m)

    res = run_bass_kernel_spmd(nc, in_maps, core_ids=list(range(B)))
    out = np.stack([r["out"] for r in res.results], axis=0)
    return out.reshape(B, n_chunks, N, S, D)


# revision 28
# speedup vs baseline: 60.1387x; 60.1387x over previous
"""Trainium2 Bass kernel for nn_CrossAttention (8-core data-parallel over batch).

Reference math (per batch b, chunk c):
  en = LayerNorm(e) ; q = en@Wq+bq ; k = h@Wk+bk ; v = h@Wv+bv
  attn = softmax(q@k^T / 8) ; o = attn@v ; out = o@Wo + bo + e

Host-side folding:  q = ((e-mu)*rstd) @ (ln_g[:,None]*Wq/8) + (ln_b@Wq+bq)/8
so the on-chip LN is just (e-mu)*rstd.

On-chip dataflow (all matmuls fp32r):
  - activations transposed to d-major ("T") layout via PE transposes
  - qT projection with weights as stationary lhsT; biases added by K=1
    piggyback matmuls (ones-row x bias-row) inside each accumulation group
  - kT and v stored in block-diagonal head-pair layout so scores and AV are
    single full-array K=128 matmuls per pair (fp32r cannot write PSUM at a
    column offset, which rules out tile_position pairing)
  - softmax denominators via one accumulated E2 matmul -> psum [12, 256];
    1/den partition-broadcast via K=12 matmul with R_p selector constants;
    normalization applied at AV-psum evacuation (oT = pav * bcast)
  - O-proj, PE transpose back to row-major with residual fused into the
    evacuation, store from the e tile (never fp32r-consumed)
  - two-stage software pipeline: stage A (load/LN/xT/Q/vbd) of chunk c is
    emitted before stage B (attention/O/store) of chunk c-1
"""

import numpy as np

B, C, N, S, D = 8, 32, 4, 64, 768
NH, DK = 12, 64
R = N * S          # 256 rows per chunk
KO = D // 128      # 6 partition blocks of d
NP = 6             # head pairs
LN_EPS = 1e-5
GROUP = 4          # chunks per h/kv batch group

_prog_cache = {}


def _build(n_chunks):
    import concourse.bass as bass
    import concourse.tile as tile
    from concourse import bacc, mybir
    from contextlib import ExitStack

    F32 = mybir.dt.float32
    F32R = mybir.dt.float32r
    AF = mybir.ActivationFunctionType
    ALU = mybir.AluOpType

    nc = bacc.Bacc()

    d_e = nc.dram_tensor("e", [n_chunks, R, D], F32, kind="ExternalInput")
    d_h = nc.dram_tensor("h", [n_chunks, S, D], F32R, kind="ExternalInput")
    d_wq = nc.dram_tensor("wq", [KO, 128, D], F32R, kind="ExternalInput")
    d_wk = nc.dram_tensor("wk", [KO, 128, D], F32R, kind="ExternalInput")
    d_wv = nc.dram_tensor("wv", [KO, 128, D], F32R, kind="ExternalInput")
    d_wo = nc.dram_tensor("wo", [KO, 128, D], F32R, kind="ExternalInput")
    d_bqc = nc.dram_tensor("bqc", [128, KO], F32, kind="ExternalInput")
    d_bkc = nc.dram_tensor("bkc", [128, KO], F32, kind="ExternalInput")
    d_boc = nc.dram_tensor("boc", [128, KO], F32, kind="ExternalInput")
    d_e2c = nc.dram_tensor("e2c", [128, NP, NH], F32R, kind="ExternalInput")
    d_rpc = nc.dram_tensor("rpc", [NH, NP, 128], F32R, kind="ExternalInput")
    d_id = nc.dram_tensor("ident", [128, 128], F32R, kind="ExternalInput")
    d_zf = nc.dram_tensor("zf", [128, 1], F32R, kind="ExternalInput")
    d_ones = nc.dram_tensor("ones", [1, 512], F32R, kind="ExternalInput")
    d_bvrr = nc.dram_tensor("bvrr", [1, D], F32R, kind="ExternalInput")
    d_out = nc.dram_tensor("out", [n_chunks, R, D], F32, kind="ExternalOutput")

    with ExitStack() as ctx:
        tc = ctx.enter_context(tile.TileContext(nc))
        consts = ctx.enter_context(tc.tile_pool(name="consts", bufs=1))
        e_pool = ctx.enter_context(tc.tile_pool(name="e_pool", bufs=2))
        x_pool = ctx.enter_context(tc.tile_pool(name="x_pool", bufs=2))
        xT_pool = ctx.enter_context(tc.tile_pool(name="xT_pool", bufs=2))
        q_pool = ctx.enter_context(tc.tile_pool(name="q_pool", bufs=2))
        exp_pool = ctx.enter_context(tc.tile_pool(name="exp_pool", bufs=2))
        oT_pool = ctx.enter_context(tc.tile_pool(name="oT_pool", bufs=1))
        fT_pool = ctx.enter_context(tc.tile_pool(name="fT_pool", bufs=1))
        vd_pool = ctx.enter_context(tc.tile_pool(name="vd_pool", bufs=2))
        grp_pool = ctx.enter_context(tc.tile_pool(name="grp_pool", bufs=1))
        kt_pool = ctx.enter_context(tc.tile_pool(name="kt_pool", bufs=1))
        v2_pool = ctx.enter_context(tc.tile_pool(name="v2_pool", bufs=2))
        st_pool = ctx.enter_context(tc.tile_pool(name="st_pool", bufs=2))
        ps_mmA = ctx.enter_context(tc.tile_pool(name="ps_mmA", bufs=2, space="PSUM"))
        ps_mmB = ctx.enter_context(tc.tile_pool(name="ps_mmB", bufs=2, space="PSUM"))
        ps_s = ctx.enter_context(tc.tile_pool(name="ps_s", bufs=1, space="PSUM"))
        ps_ab = ctx.enter_context(tc.tile_pool(name="ps_ab", bufs=2, space="PSUM"))
        ps_d = ctx.enter_context(tc.tile_pool(name="ps_d", bufs=1, space="PSUM"))

        # ---- constants ----
        wq = consts.tile([128, KO, D], F32R)
        wk = consts.tile([128, KO, D], F32R)
        wv = consts.tile([128, KO, D], F32R)
        wo = consts.tile([128, KO, D], F32R)
        nc.sync.dma_start(wq[:], d_wq[:].rearrange("k p d -> p k d"))
        nc.sync.dma_start(wk[:], d_wk[:].rearrange("k p d -> p k d"))
        nc.sync.dma_start(wv[:], d_wv[:].rearrange("k p d -> p k d"))
        nc.sync.dma_start(wo[:], d_wo[:].rearrange("k p d -> p k d"))
        bqc = consts.tile([128, KO], F32)
        bkc = consts.tile([128, KO], F32)
        boc = consts.tile([128, KO], F32)
        nc.sync.dma_start(bqc[:], d_bqc[:])
        nc.sync.dma_start(bkc[:], d_bkc[:])
        nc.sync.dma_start(boc[:], d_boc[:])
        e2c = consts.tile([128, NP, NH], F32R)
        rpc = consts.tile([NH, NP, 128], F32R)
        ident = consts.tile([128, 128], F32R)
        zf = consts.tile([128, 1], F32R)
        ones = consts.tile([1, 512], F32R)
        bvrr = consts.tile([1, D], F32R)
        nc.sync.dma_start(e2c[:], d_e2c[:])
        nc.sync.dma_start(rpc[:], d_rpc[:])
        nc.sync.dma_start(ident[:], d_id[:])
        nc.sync.dma_start(zf[:], d_zf[:])
        nc.sync.dma_start(ones[:], d_ones[:])
        nc.sync.dma_start(bvrr[:], d_bvrr[:])
        eps_t = consts.tile([128, 1], F32)
        nc.vector.memset(eps_t[:], LN_EPS)

        def group_phase(g):
            # h load (halves), transpose, K/V projections
            hT4 = grp_pool.tile([128, KO, GROUP * S], F32R, tag="hT4")
            for hh in range(2):
                h2 = grp_pool.tile([S, 2, D], F32R, tag="h2")
                nc.sync.dma_start(
                    h2[:], d_h[g * GROUP + 2 * hh:g * GROUP + 2 * hh + 2]
                    .rearrange("c j d -> j c d"))
                for c2 in range(2):
                    cc = 2 * hh + c2
                    for k0, kn in ((0, 4), (4, 2)):
                        ptq = ps_mmA.tile([128, 4, 128], F32R, tag="mmA")
                        for i in range(kn):
                            nc.tensor.transpose(
                                ptq[:, i, 0:S],
                                h2[:, c2, (k0 + i) * 128:(k0 + i + 1) * 128],
                                ident[0:S, 0:S])
                        nc.vector.tensor_copy(
                            hT4[:, k0:k0 + kn, cc * S:(cc + 1) * S],
                            ptq[:, 0:kn, 0:S])

            # kT in block-diagonal pair layout
            kbd = kt_pool.tile([128, NP, GROUP, 128], F32R, tag="kbd")
            nc.gpsimd.tensor_copy(
                kbd[:], zf[:, None, None, 0:1].to_broadcast(
                    [128, NP, GROUP, 128]))
            for mo in range(KO):
                pk = ps_mmA.tile([128, 512], F32, tag="mmA")
                for ko in range(KO):
                    nc.tensor.matmul(
                        pk[:, 0:GROUP * S],
                        wk[:, ko, mo * 128:(mo + 1) * 128],
                        hT4[:, ko, :],
                        start=(ko == 0), stop=(ko == KO - 1))
                pkv = pk[:, 0:GROUP * S].rearrange("p (c j) -> p c j", c=GROUP)
                nc.vector.tensor_scalar(
                    out=kbd[0:64, mo, :, 0:S], in0=pkv[0:64],
                    scalar1=bkc[0:64, mo:mo + 1], scalar2=None, op0=ALU.add)
                nc.vector.tensor_scalar(
                    out=kbd[64:128, mo, :, S:128], in0=pkv[64:128],
                    scalar1=bkc[64:128, mo:mo + 1], scalar2=None, op0=ALU.add)

            v2 = []
            for st in range(GROUP // 2):
                v2t = v2_pool.tile([128, D], F32R, tag="v2")
                for n0, ns in ((0, 512), (512, 256)):
                    pv = ps_mmA.tile([128, 512], F32, tag="mmA")
                    for ko in range(KO):
                        nc.tensor.matmul(
                            pv[:, 0:ns],
                            hT4[:, ko, st * 128:(st + 1) * 128],
                            wv[:, ko, n0:n0 + ns],
                            start=(ko == 0), stop=False)
                    nc.tensor.matmul(
                        pv[:, 0:ns], ones[:, 0:128],
                        bvrr[:, n0:n0 + ns], start=False, stop=True)
                    nc.vector.tensor_copy(v2t[:, n0:n0 + ns], pv[:, 0:ns])
                v2.append(v2t)
            return kbd, v2

        def stage_a(c, hctx):
            kbd, v2 = hctx
            cc = c % GROUP
            # ---- load e, LayerNorm stats + apply ----
            e_sb = e_pool.tile([128, 2, D], F32, tag="e")
            nc.sync.dma_start(
                e_sb[:], d_e[c].rearrange("(t p) d -> p t d", p=128))

            stats = st_pool.tile([128, 2, 3, 6], F32, tag="stats")
            mv = st_pool.tile([128, 2, 2], F32, tag="mv")
            rstd = st_pool.tile([128, 2], F32, tag="rstd")
            x_sb = x_pool.tile([128, 2, D], F32R, tag="x")
            for t in range(2):
                esl = e_sb[:, t, :].rearrange("p (s f) -> p s f", s=3)
                for sg in range(3):
                    nc.vector.bn_stats(stats[:, t, sg, :], esl[:, sg, :])
                nc.vector.bn_aggr(mv[:, t, :], stats[:, t, :, :])
            # rstd = rsqrt(var + eps) via bit-hack + 2 Newton steps (DVE only,
            # keeps Sqrt out of ACT so no act-table reloads)
            I32 = mybir.dt.int32
            v1 = st_pool.tile([128, 2], F32, tag="v1")
            y = st_pool.tile([128, 2], F32, tag="y")
            tmp = st_pool.tile([128, 2], F32, tag="tmp")
            nc.vector.tensor_scalar(
                out=v1[:], in0=mv[:, :, 1], scalar1=float(LN_EPS), scalar2=None,
                op0=ALU.add)
            nc.vector.tensor_scalar(
                out=y[:].bitcast(I32), in0=v1[:].bitcast(I32), scalar1=1,
                scalar2=None, op0=ALU.logical_shift_right)
            nc.vector.tensor_scalar(
                out=y[:].bitcast(I32), in0=y[:].bitcast(I32), scalar1=-1,
                scalar2=0x5F3759DF, op0=ALU.mult, op1=ALU.add)
            for _ in range(2):
                nc.vector.tensor_tensor(
                    out=tmp[:], in0=y[:], in1=y[:], op=ALU.mult)
                nc.vector.tensor_tensor(
                    out=tmp[:], in0=tmp[:], in1=v1[:], op=ALU.mult)
                nc.vector.tensor_scalar(
                    out=tmp[:], in0=tmp[:], scalar1=-0.5, scalar2=1.5,
                    op0=ALU.mult, op1=ALU.add)
                nc.vector.tensor_tensor(
                    out=rstd[:], in0=y[:], in1=tmp[:], op=ALU.mult)
                nc.vector.tensor_copy(y[:], rstd[:])
            for t in range(2):
                nc.gpsimd.tensor_scalar(
                    out=x_sb[:, t, :], in0=e_sb[:, t, :],
                    scalar1=mv[:, t, 0:1], scalar2=rstd[:, t:t + 1],
                    op0=ALU.subtract, op1=ALU.mult)

            # ---- transpose x to d-major ----
            xT = xT_pool.tile([128, KO, R], F32R, tag="xT")
            for ko2 in range(KO // 2):
                pt4 = ps_mmA.tile([128, 4, 128], F32R, tag="mmA")
                for i in range(2):
                    for t in range(2):
                        nc.tensor.transpose(
                            pt4[:, 2 * i + t, :],
                            x_sb[:, t, (2 * ko2 + i) * 128:(2 * ko2 + i + 1) * 128],
                            ident[:])
                nc.vector.tensor_copy(xT[:, 2 * ko2:2 * ko2 + 2, :], pt4[:])

            # ---- Q projection ----
            qT = q_pool.tile([128, KO, R], F32R, tag="qT")
            for mo in range(KO):
                pq = ps_mmA.tile([128, 512], F32, tag="mmA")
                for ko in range(KO):
                    nc.tensor.matmul(
                        pq[:, 0:R], wq[:, ko, mo * 128:(mo + 1) * 128],
                        xT[:, ko, :], start=(ko == 0), stop=(ko == KO - 1))
                nc.scalar.activation(
                    qT[:, mo, :], pq[:, 0:R], AF.Identity,
                    bias=bqc[:, mo:mo + 1], scale=1.0)

            # ---- v in block-diagonal pair layout ----
            v2t = v2[cc // 2]
            pa = 64 * (cc % 2)
            vbd = vd_pool.tile([128, NP, 128], F32R, tag="vbd")
            nc.gpsimd.tensor_copy(
                vbd[:], zf[:, None, 0:1].to_broadcast([128, NP, 128]))
            v2v = v2t[pa:pa + 64, :].rearrange(
                "p (np two dk) -> p np two dk", np=NP, two=2)
            nc.gpsimd.tensor_copy(vbd[0:64, :, 0:DK], v2v[:, :, 0, :])
            nc.gpsimd.tensor_copy(vbd[64:128, :, DK:128], v2v[:, :, 1, :])
            return (c, e_sb, x_sb, xT, qT, vbd)

        def stage_b(actx, hctx):
            c, e_sb, x_sb, xT, qT, vbd = actx
            kbd, v2 = hctx
            cc = c % GROUP

            # ---- attention (head-pairs processed two at a time) ----
            expT = exp_pool.tile([128, NP, R], F32R, tag="expT")
            pden = ps_d.tile([NH, R], F32, tag="den")
            for p2 in range(0, NP, 2):
                pscr = ps_s.tile([128, 2, R], F32, tag="s")
                for i in range(2):
                    nc.tensor.matmul(
                        pscr[:, i, :], kbd[:, p2 + i, cc, :], qT[:, p2 + i, :],
                        start=True, stop=True)
                nc.scalar.activation(
                    expT[:, p2:p2 + 2, :], pscr[:], AF.Exp, bias=0.0, scale=1.0)
                for i in range(2):
                    p = p2 + i
                    nc.tensor.matmul(
                        pden[:], e2c[:, p, :], expT[:, p, :],
                        start=(p == 0), stop=(p == NP - 1),
                        skip_group_check=True)

            recip = st_pool.tile([NH, R], F32R, tag="recip")
            with nc.allow_low_precision(reason="fp32r softmax denom"):
                nc.vector.reciprocal(recip[:], pden[:])

            oT = oT_pool.tile([128, KO, R], F32R, tag="oT")
            for p2 in range(0, NP, 2):
                pav = ps_ab.tile([128, 2, R], F32, tag="ab")
                pbc = ps_ab.tile([128, 2, R], F32, tag="ab")
                for i in range(2):
                    nc.tensor.matmul(
                        pav[:, i, :], vbd[:, p2 + i, :], expT[:, p2 + i, :],
                        start=True, stop=True)
                    nc.tensor.matmul(
                        pbc[:, i, :], rpc[:, p2 + i, :], recip[:],
                        start=True, stop=True)
                bc_sb = st_pool.tile([128, 2, R], F32, tag="bcsb")
                nc.scalar.copy(bc_sb[:], pbc[:])
                nc.vector.tensor_tensor(
                    out=oT[:, p2:p2 + 2, :], in0=pav[:], in1=bc_sb[:],
                    op=ALU.mult)

            # ---- O projection ----
            fT = fT_pool.tile([128, KO, R], F32R, tag="fT")
            for mo in range(KO):
                pf = ps_mmB.tile([128, 512], F32, tag="mmB")
                for ko in range(KO):
                    nc.tensor.matmul(
                        pf[:, 0:R], wo[:, ko, mo * 128:(mo + 1) * 128],
                        oT[:, ko, :], start=(ko == 0), stop=(ko == KO - 1))
                nc.scalar.activation(
                    fT[:, mo, :], pf[:, 0:R], AF.Identity,
                    bias=boc[:, mo:mo + 1], scale=1.0)

            # ---- transpose back + residual, store ----
            for t in range(2):
                for m0, mn in ((0, 4), (4, 2)):
                    ptq = ps_mmB.tile([128, 4, 128], F32R, tag="mmB")
                    for i in range(mn):
                        nc.tensor.transpose(
                            ptq[:, i, :], fT[:, m0 + i, t * 128:(t + 1) * 128],
                            ident[:])
                    nc.vector.tensor_tensor(
                        out=e_sb[:, t, m0 * 128:(m0 + mn) * 128],
                        in0=ptq[:, 0:mn, :].bitcast(F32),
                        in1=e_sb[:, t, m0 * 128:(m0 + mn) * 128],
                        op=ALU.add)
            nc.sync.dma_start(
                d_out[c].rearrange("(t p) d -> p t d", p=128), e_sb[:])

        # ---- software-pipelined driver: A(c+1) emitted ahead of B(c) ----
        n_groups = n_chunks // GROUP
        pending = None
        for g in range(n_groups):
            hctx = group_phase(g)
            for cc in range(GROUP):
                actx = stage_a(g * GROUP + cc, hctx)
                if pending is not None:
                    stage_b(*pending)
                pending = (actx, hctx)
        if pending is not None:
            stage_b(*pending)

    nc.compile()
    return nc


def _prep_consts(Wq, bq, Wk, bk, Wv, bv, Wo, bo, ln_g, ln_b):
    scale = 1.0 / np.sqrt(DK)
    Wq_eff = (ln_g[:, None] * Wq) * scale
    bq_eff = (ln_b @ Wq + bq) * scale

    def wl(w):
        return np.ascontiguousarray(w.reshape(KO, 128, D), dtype=np.float32)

    e2c = np.zeros((128, NP, NH), dtype=np.float32)
    for p in range(NP):
        e2c[0:64, p, 2 * p] = 1.0
        e2c[64:128, p, 2 * p + 1] = 1.0
    rpc = np.zeros((NH, NP, 128), dtype=np.float32)
    for p in range(NP):
        rpc[2 * p, p, 0:64] = 1.0
        rpc[2 * p + 1, p, 64:128] = 1.0

    return {
        "wq": wl(Wq_eff), "wk": wl(Wk), "wv": wl(Wv), "wo": wl(Wo),
        "e2c": e2c, "rpc": rpc, "zf": np.zeros((128, 1), dtype=np.float32),
        "ones": np.ones((1, 512), dtype=np.float32),
        "bqc": np.ascontiguousarray(bq_eff.reshape(KO, 128).T, dtype=np.float32),
        "bkc": np.ascontiguousarray(bk.reshape(KO, 128).T, dtype=np.float32),
        "boc": np.ascontiguousarray(bo.reshape(KO, 128).T, dtype=np.float32),
        "bvrr": np.ascontiguousarray(bv.reshape(1, D), dtype=np.float32),
        "ident": np.eye(128, dtype=np.float32),
    }


def kernel(e, h, Wq, bq, Wk, bk, Wv, bv, Wo, bo, ln_g, ln_b):
    from concourse.bass_utils import run_bass_kernel_spmd

    e = np.asarray(e, dtype=np.float32)
    h = np.asarray(h, dtype=np.float32)
    n_chunks = e.shape[1]

    if n_chunks not in _prog_cache:
        _prog_cache[n_chunks] = _build(n_chunks)
    nc = _prog_cache[n_chunks]

    consts = _prep_consts(
        np.asarray(Wq, np.float32), np.asarray(bq, np.float32),
        np.asarray(Wk, np.float32), np.asarray(bk, np.float32),
        np.asarray(Wv, np.float32), np.asarray(bv, np.float32),
        np.asarray(Wo, np.float32), np.asarray(bo, np.float32),
        np.asarray(ln_g, np.float32), np.asarray(ln_b, np.float32))

    in_maps = []
    for b in range(B):
        m = dict(consts)
        m["e"] = np.ascontiguousarray(e[b].reshape(n_chunks, R, D))
        m["h"] = np.ascontiguousarray(h[b])
        in_maps.append(m)

    res = run_bass_kernel_spmd(nc, in_maps, core_ids=list(range(B)))
    out = np.stack([r["out"] for r in res.results], axis=0)
    return out.reshape(B, n_chunks, N, S, D)


# revision 35
# speedup vs baseline: 61.4308x; 1.0215x over previous
"""Trainium2 Bass kernel for nn_CrossAttention (8-core data-parallel over batch).

Reference math (per batch b, chunk c):
  en = LayerNorm(e) ; q = en@Wq+bq ; k = h@Wk+bk ; v = h@Wv+bv
  attn = softmax(q@k^T / 8) ; o = attn@v ; out = o@Wo + bo + e

Host-side folding:  q = ((e-mu)*rstd) @ (ln_g[:,None]*Wq/8) + (ln_b@Wq+bq)/8
so the on-chip LN is just (e-mu)*rstd.

On-chip dataflow (all matmuls fp32r):
  - activations transposed to d-major ("T") layout via PE transposes
  - qT projection with weights as stationary lhsT; biases added by K=1
    piggyback matmuls (ones-row x bias-row) inside each accumulation group
  - kT and v stored in block-diagonal head-pair layout so scores and AV are
    single full-array K=128 matmuls per pair (fp32r cannot write PSUM at a
    column offset, which rules out tile_position pairing)
  - softmax denominators via one accumulated E2 matmul -> psum [12, 256];
    1/den partition-broadcast via K=12 matmul with R_p selector constants;
    normalization applied at AV-psum evacuation (oT = pav * bcast)
  - O-proj, PE transpose back to row-major with residual fused into the
    evacuation, store from the e tile (never fp32r-consumed)
  - two-stage software pipeline: stage A (load/LN/xT/Q/vbd) of chunk c is
    emitted before stage B (attention/O/store) of chunk c-1
"""

import numpy as np

B, C, N, S, D = 8, 32, 4, 64, 768
NH, DK = 12, 64
R = N * S          # 256 rows per chunk
KO = D // 128      # 6 partition blocks of d
NP = 6             # head pairs
LN_EPS = 1e-5
GROUP = 4          # chunks per h/kv batch group

_prog_cache = {}


def _build(n_chunks):
    import concourse.bass as bass
    import concourse.tile as tile
    from concourse import bacc, mybir
    from contextlib import ExitStack

    F32 = mybir.dt.float32
    F32R = mybir.dt.float32r
    AF = mybir.ActivationFunctionType
    ALU = mybir.AluOpType

    nc = bacc.Bacc()

    d_e = nc.dram_tensor("e", [n_chunks, R, D], F32, kind="ExternalInput")
    d_h = nc.dram_tensor("h", [n_chunks, S, D], F32R, kind="ExternalInput")
    d_wq = nc.dram_tensor("wq", [KO, 128, D], F32R, kind="ExternalInput")
    d_wk = nc.dram_tensor("wk", [KO, 128, D], F32R, kind="ExternalInput")
    d_wv = nc.dram_tensor("wv", [KO, 128, D], F32R, kind="ExternalInput")
    d_wo = nc.dram_tensor("wo", [KO, 128, D], F32R, kind="ExternalInput")
    d_bqc = nc.dram_tensor("bqc", [128, KO], F32, kind="ExternalInput")
    d_bkc = nc.dram_tensor("bkc", [128, KO], F32, kind="ExternalInput")
    d_boc = nc.dram_tensor("boc", [128, KO], F32, kind="ExternalInput")
    d_e2c = nc.dram_tensor("e2c", [128, NP, NH], F32R, kind="ExternalInput")
    d_rpc = nc.dram_tensor("rpc", [NH, NP, 128], F32R, kind="ExternalInput")
    d_id = nc.dram_tensor("ident", [128, 128], F32R, kind="ExternalInput")
    d_zf = nc.dram_tensor("zf", [128, 1], F32R, kind="ExternalInput")
    d_ones = nc.dram_tensor("ones", [1, 512], F32R, kind="ExternalInput")
    d_bvrr = nc.dram_tensor("bvrr", [1, D], F32R, kind="ExternalInput")
    d_out = nc.dram_tensor("out", [n_chunks, R, D], F32, kind="ExternalOutput")

    with ExitStack() as ctx:
        tc = ctx.enter_context(tile.TileContext(nc))
        consts = ctx.enter_context(tc.tile_pool(name="consts", bufs=1))
        e_pool = ctx.enter_context(tc.tile_pool(name="e_pool", bufs=2))
        x_pool = ctx.enter_context(tc.tile_pool(name="x_pool", bufs=2))
        xT_pool = ctx.enter_context(tc.tile_pool(name="xT_pool", bufs=2))
        q_pool = ctx.enter_context(tc.tile_pool(name="q_pool", bufs=2))
        exp_pool = ctx.enter_context(tc.tile_pool(name="exp_pool", bufs=2))
        oT_pool = ctx.enter_context(tc.tile_pool(name="oT_pool", bufs=1))
        fT_pool = ctx.enter_context(tc.tile_pool(name="fT_pool", bufs=1))
        vd_pool = ctx.enter_context(tc.tile_pool(name="vd_pool", bufs=2))
        grp_pool = ctx.enter_context(tc.tile_pool(name="grp_pool", bufs=1))
        kt_pool = ctx.enter_context(tc.tile_pool(name="kt_pool", bufs=1))
        v2_pool = ctx.enter_context(tc.tile_pool(name="v2_pool", bufs=2))
        st_pool = ctx.enter_context(tc.tile_pool(name="st_pool", bufs=2))
        ps_mmA = ctx.enter_context(tc.tile_pool(name="ps_mmA", bufs=2, space="PSUM"))
        ps_mmB = ctx.enter_context(tc.tile_pool(name="ps_mmB", bufs=2, space="PSUM"))
        ps_s = ctx.enter_context(tc.tile_pool(name="ps_s", bufs=2, space="PSUM"))
        ps_ab = ctx.enter_context(tc.tile_pool(name="ps_ab", bufs=2, space="PSUM"))

        # ---- constants ----
        wq = consts.tile([128, KO, D], F32R)
        wk = consts.tile([128, KO, D], F32R)
        wv = consts.tile([128, KO, D], F32R)
        wo = consts.tile([128, KO, D], F32R)
        nc.sync.dma_start(wq[:], d_wq[:].rearrange("k p d -> p k d"))
        nc.sync.dma_start(wk[:], d_wk[:].rearrange("k p d -> p k d"))
        nc.sync.dma_start(wv[:], d_wv[:].rearrange("k p d -> p k d"))
        nc.sync.dma_start(wo[:], d_wo[:].rearrange("k p d -> p k d"))
        bqc = consts.tile([128, KO], F32)
        bkc = consts.tile([128, KO], F32)
        boc = consts.tile([128, KO], F32)
        nc.sync.dma_start(bqc[:], d_bqc[:])
        nc.sync.dma_start(bkc[:], d_bkc[:])
        nc.sync.dma_start(boc[:], d_boc[:])
        e2c = consts.tile([128, NP, NH], F32R)
        rpc = consts.tile([NH, NP, 128], F32R)
        ident = consts.tile([128, 128], F32R)
        zf = consts.tile([128, 1], F32R)
        ones = consts.tile([1, 512], F32R)
        bvrr = consts.tile([1, D], F32R)
        nc.sync.dma_start(e2c[:], d_e2c[:])
        nc.sync.dma_start(rpc[:], d_rpc[:])
        nc.sync.dma_start(ident[:], d_id[:])
        nc.sync.dma_start(zf[:], d_zf[:])
        nc.sync.dma_start(ones[:], d_ones[:])
        nc.sync.dma_start(bvrr[:], d_bvrr[:])
        eps_t = consts.tile([128, 1], F32)
        nc.vector.memset(eps_t[:], LN_EPS)

        def group_phase(g):
            # h load (halves), transpose, K/V projections
            hT4 = grp_pool.tile([128, KO, GROUP * S], F32R, tag="hT4")
            for hh in range(2):
                h2 = grp_pool.tile([S, 2, D], F32R, tag="h2")
                nc.sync.dma_start(
                    h2[:], d_h[g * GROUP + 2 * hh:g * GROUP + 2 * hh + 2]
                    .rearrange("c j d -> j c d"))
                for c2 in range(2):
                    cc = 2 * hh + c2
                    for k0, kn in ((0, 4), (4, 2)):
                        ptq = ps_mmA.tile([128, 4, 128], F32R, tag="mmA")
                        for i in range(kn):
                            nc.tensor.transpose(
                                ptq[:, i, 0:S],
                                h2[:, c2, (k0 + i) * 128:(k0 + i + 1) * 128],
                                ident[0:S, 0:S])
                        nc.vector.tensor_copy(
                            hT4[:, k0:k0 + kn, cc * S:(cc + 1) * S],
                            ptq[:, 0:kn, 0:S])

            # kT in block-diagonal pair layout
            kbd = kt_pool.tile([128, NP, GROUP, 128], F32R, tag="kbd")
            nc.gpsimd.tensor_copy(
                kbd[:], zf[:, None, None, 0:1].to_broadcast(
                    [128, NP, GROUP, 128]))
            for mo in range(KO):
                pk = ps_mmA.tile([128, 512], F32, tag="mmA")
                for ko in range(KO):
                    nc.tensor.matmul(
                        pk[:, 0:GROUP * S],
                        wk[:, ko, mo * 128:(mo + 1) * 128],
                        hT4[:, ko, :],
                        start=(ko == 0), stop=(ko == KO - 1))
                pkv = pk[:, 0:GROUP * S].rearrange("p (c j) -> p c j", c=GROUP)
                nc.vector.tensor_scalar(
                    out=kbd[0:64, mo, :, 0:S], in0=pkv[0:64],
                    scalar1=bkc[0:64, mo:mo + 1], scalar2=None, op0=ALU.add)
                nc.vector.tensor_scalar(
                    out=kbd[64:128, mo, :, S:128], in0=pkv[64:128],
                    scalar1=bkc[64:128, mo:mo + 1], scalar2=None, op0=ALU.add)

            v2 = []
            for st in range(GROUP // 2):
                v2t = v2_pool.tile([128, D], F32R, tag="v2")
                for n0, ns in ((0, 512), (512, 256)):
                    pv = ps_mmA.tile([128, 512], F32, tag="mmA")
                    for ko in range(KO):
                        nc.tensor.matmul(
                            pv[:, 0:ns],
                            hT4[:, ko, st * 128:(st + 1) * 128],
                            wv[:, ko, n0:n0 + ns],
                            start=(ko == 0), stop=False)
                    nc.tensor.matmul(
                        pv[:, 0:ns], ones[:, 0:128],
                        bvrr[:, n0:n0 + ns], start=False, stop=True)
                    nc.vector.tensor_copy(v2t[:, n0:n0 + ns], pv[:, 0:ns])
                v2.append(v2t)
            return kbd, v2

        def stage_a(c, hctx):
            kbd, v2 = hctx
            cc = c % GROUP
            # ---- load e, LayerNorm stats + apply ----
            e_sb = e_pool.tile([128, 2, D], F32, tag="e")
            nc.sync.dma_start(
                e_sb[:], d_e[c].rearrange("(t p) d -> p t d", p=128))

            stats = st_pool.tile([128, 2, 3, 6], F32, tag="stats")
            mv = st_pool.tile([128, 2, 2], F32, tag="mv")
            rstd = st_pool.tile([128, 2], F32, tag="rstd")
            x_sb = x_pool.tile([128, 2, D], F32R, tag="x")
            for t in range(2):
                esl = e_sb[:, t, :].rearrange("p (s f) -> p s f", s=3)
                for sg in range(3):
                    nc.vector.bn_stats(stats[:, t, sg, :], esl[:, sg, :])
                nc.vector.bn_aggr(mv[:, t, :], stats[:, t, :, :])
            # rstd = rsqrt(var + eps) via bit-hack + 2 Newton steps (DVE only,
            # keeps Sqrt out of ACT so no act-table reloads)
            I32 = mybir.dt.int32
            v1 = st_pool.tile([128, 2], F32, tag="v1")
            y = st_pool.tile([128, 2], F32, tag="y")
            tmp = st_pool.tile([128, 2], F32, tag="tmp")
            nc.vector.tensor_scalar(
                out=v1[:], in0=mv[:, :, 1], scalar1=float(LN_EPS), scalar2=None,
                op0=ALU.add)
            nc.vector.tensor_scalar(
                out=y[:].bitcast(I32), in0=v1[:].bitcast(I32), scalar1=1,
                scalar2=None, op0=ALU.logical_shift_right)
            nc.vector.tensor_scalar(
                out=y[:].bitcast(I32), in0=y[:].bitcast(I32), scalar1=-1,
                scalar2=0x5F3759DF, op0=ALU.mult, op1=ALU.add)
            for _ in range(2):
                nc.vector.tensor_tensor(
                    out=tmp[:], in0=y[:], in1=y[:], op=ALU.mult)
                nc.vector.tensor_tensor(
                    out=tmp[:], in0=tmp[:], in1=v1[:], op=ALU.mult)
                nc.vector.tensor_scalar(
                    out=tmp[:], in0=tmp[:], scalar1=-0.5, scalar2=1.5,
                    op0=ALU.mult, op1=ALU.add)
                nc.vector.tensor_tensor(
                    out=rstd[:], in0=y[:], in1=tmp[:], op=ALU.mult)
                nc.vector.tensor_copy(y[:], rstd[:])
            for t in range(2):
                nc.gpsimd.tensor_scalar(
                    out=x_sb[:, t, :], in0=e_sb[:, t, :],
                    scalar1=mv[:, t, 0:1], scalar2=rstd[:, t:t + 1],
                    op0=ALU.subtract, op1=ALU.mult)

            # ---- transpose x to d-major ----
            xT = xT_pool.tile([128, KO, R], F32R, tag="xT")
            for ko2 in range(KO // 2):
                pt4 = ps_mmA.tile([128, 4, 128], F32R, tag="mmA")
                for i in range(2):
                    for t in range(2):
                        nc.tensor.transpose(
                            pt4[:, 2 * i + t, :],
                            x_sb[:, t, (2 * ko2 + i) * 128:(2 * ko2 + i + 1) * 128],
                            ident[:])
                nc.vector.tensor_copy(xT[:, 2 * ko2:2 * ko2 + 2, :], pt4[:])

            # ---- Q projection ----
            qT = q_pool.tile([128, KO, R], F32R, tag="qT")
            for mo in range(KO):
                pq = ps_mmA.tile([128, 512], F32, tag="mmA")
                for ko in range(KO):
                    nc.tensor.matmul(
                        pq[:, 0:R], wq[:, ko, mo * 128:(mo + 1) * 128],
                        xT[:, ko, :], start=(ko == 0), stop=(ko == KO - 1))
                nc.scalar.activation(
                    qT[:, mo, :], pq[:, 0:R], AF.Identity,
                    bias=bqc[:, mo:mo + 1], scale=1.0)

            # ---- v in block-diagonal pair layout ----
            v2t = v2[cc // 2]
            pa = 64 * (cc % 2)
            vbd = vd_pool.tile([128, NP, 128], F32R, tag="vbd")
            nc.gpsimd.tensor_copy(
                vbd[:], zf[:, None, 0:1].to_broadcast([128, NP, 128]))
            v2v = v2t[pa:pa + 64, :].rearrange(
                "p (np two dk) -> p np two dk", np=NP, two=2)
            nc.gpsimd.tensor_copy(vbd[0:64, :, 0:DK], v2v[:, :, 0, :])
            nc.gpsimd.tensor_copy(vbd[64:128, :, DK:128], v2v[:, :, 1, :])
            return (c, e_sb, x_sb, xT, qT, vbd)

        def stage_b(actx, hctx):
            c, e_sb, x_sb, xT, qT, vbd = actx
            kbd, v2 = hctx
            cc = c % GROUP

            # ---- attention (head-pairs processed two at a time) ----
            expT = exp_pool.tile([128, NP, R], F32R, tag="expT")
            pden_t = ps_s.tile([128, 2, R], F32, tag="s", name="pden_t")
            pden = pden_t[0:NH, 0, :]
            for p2 in range(0, NP, 2):
                pscr = ps_s.tile([128, 2, R], F32, tag="s")
                for i in range(2):
                    nc.tensor.matmul(
                        pscr[:, i, :], kbd[:, p2 + i, cc, :], qT[:, p2 + i, :],
                        start=True, stop=True)
                nc.scalar.activation(
                    expT[:, p2:p2 + 2, :], pscr[:], AF.Exp, bias=0.0, scale=1.0)
            for p in range(NP):
                nc.tensor.matmul(
                    pden[:], e2c[:, p, :], expT[:, p, :],
                    start=(p == 0), stop=(p == NP - 1),
                    skip_group_check=True)

            recip = st_pool.tile([NH, R], F32R, tag="recip")
            with nc.allow_low_precision(reason="fp32r softmax denom"):
                nc.vector.reciprocal(recip[:], pden[:])

            oT = oT_pool.tile([128, KO, R], F32R, tag="oT")
            for p2 in range(0, NP, 2):
                pav = ps_ab.tile([128, 2, R], F32, tag="ab")
                pbc = ps_ab.tile([128, 2, R], F32, tag="ab")
                for i in range(2):
                    nc.tensor.matmul(
                        pav[:, i, :], vbd[:, p2 + i, :], expT[:, p2 + i, :],
                        start=True, stop=True)
                    nc.tensor.matmul(
                        pbc[:, i, :], rpc[:, p2 + i, :], recip[:],
                        start=True, stop=True)
                bc_sb = st_pool.tile([128, 2, R], F32, tag="bcsb")
                nc.scalar.copy(bc_sb[:], pbc[:])
                nc.vector.tensor_tensor(
                    out=oT[:, p2:p2 + 2, :], in0=pav[:], in1=bc_sb[:],
                    op=ALU.mult)

            # ---- O projection ----
            fT = fT_pool.tile([128, KO, R], F32R, tag="fT")
            for mo in range(KO):
                pf = ps_mmB.tile([128, 512], F32, tag="mmB")
                for ko in range(KO):
                    nc.tensor.matmul(
                        pf[:, 0:R], wo[:, ko, mo * 128:(mo + 1) * 128],
                        oT[:, ko, :], start=(ko == 0), stop=(ko == KO - 1))
                nc.scalar.activation(
                    fT[:, mo, :], pf[:, 0:R], AF.Identity,
                    bias=boc[:, mo:mo + 1], scale=1.0)

            # ---- transpose back + residual, store ----
            for t in range(2):
                for m0, mn in ((0, 4), (4, 2)):
                    ptq = ps_mmB.tile([128, 4, 128], F32R, tag="mmB")
                    for i in range(mn):
                        nc.tensor.transpose(
                            ptq[:, i, :], fT[:, m0 + i, t * 128:(t + 1) * 128],
                            ident[:])
                    nc.vector.tensor_tensor(
                        out=e_sb[:, t, m0 * 128:(m0 + mn) * 128],
                        in0=ptq[:, 0:mn, :].bitcast(F32),
                        in1=e_sb[:, t, m0 * 128:(m0 + mn) * 128],
                        op=ALU.add)
            nc.sync.dma_start(
                d_out[c].rearrange("(t p) d -> p t d", p=128), e_sb[:])

        # ---- software-pipelined driver: A(c+1) emitted ahead of B(c) ----
        n_groups = n_chunks // GROUP
        pending = None
        for g in range(n_groups):
            hctx = group_phase(g)
            for cc in range(GROUP):
                actx = stage_a(g * GROUP + cc, hctx)
                if pending is not None:
                    stage_b(*pending)
                pending = (actx, hctx)
        if pending is not None:
            stage_b(*pending)

    nc.compile()
    return nc


def _prep_consts(Wq, bq, Wk, bk, Wv, bv, Wo, bo, ln_g, ln_b):
    scale = 1.0 / np.sqrt(DK)
    Wq_eff = (ln_g[:, None] * Wq) * scale
    bq_eff = (ln_b @ Wq + bq) * scale

    def wl(w):
        return np.ascontiguousarray(w.reshape(KO, 128, D), dtype=np.float32)

    e2c = np.zeros((128, NP, NH), dtype=np.float32)
    for p in range(NP):
        e2c[0:64, p, 2 * p] = 1.0
        e2c[64:128, p, 2 * p + 1] = 1.0
    rpc = np.zeros((NH, NP, 128), dtype=np.float32)
    for p in range(NP):
        rpc[2 * p, p, 0:64] = 1.0
        rpc[2 * p + 1, p, 64:128] = 1.0

    return {
        "wq": wl(Wq_eff), "wk": wl(Wk), "wv": wl(Wv), "wo": wl(Wo),
        "e2c": e2c, "rpc": rpc, "zf": np.zeros((128, 1), dtype=np.float32),
        "ones": np.ones((1, 512), dtype=np.float32),
        "bqc": np.ascontiguousarray(bq_eff.reshape(KO, 128).T, dtype=np.float32),
        "bkc": np.ascontiguousarray(bk.reshape(KO, 128).T, dtype=np.float32),
        "boc": np.ascontiguousarray(bo.reshape(KO, 128).T, dtype=np.float32),
        "bvrr": np.ascontiguousarray(bv.reshape(1, D), dtype=np.float32),
        "ident": np.eye(128, dtype=np.float32),
    }


def kernel(e, h, Wq, bq, Wk, bk, Wv, bv, Wo, bo, ln_g, ln_b):
    from concourse.bass_utils import run_bass_kernel_spmd

    e = np.asarray(e, dtype=np.float32)
    h = np.asarray(h, dtype=np.float32)
    n_chunks = e.shape[1]

    if n_chunks not in _prog_cache:
        _prog_cache[n_chunks] = _build(n_chunks)
    nc = _prog_cache[n_chunks]

    consts = _prep_consts(
        np.asarray(Wq, np.float32), np.asarray(bq, np.float32),
        np.asarray(Wk, np.float32), np.asarray(bk, np.float32),
        np.asarray(Wv, np.float32), np.asarray(bv, np.float32),
        np.asarray(Wo, np.float32), np.asarray(bo, np.float32),
        np.asarray(ln_g, np.float32), np.asarray(ln_b, np.float32))

    in_maps = []
    for b in range(B):
        m = dict(consts)
        m["e"] = np.ascontiguousarray(e[b].reshape(n_chunks, R, D))
        m["h"] = np.ascontiguousarray(h[b])
        in_maps.append(m)

    res = run_bass_kernel_spmd(nc, in_maps, core_ids=list(range(B)))
    out = np.stack([r["out"] for r in res.results], axis=0)
    return out.reshape(B, n_chunks, N, S, D)


# revision 39
# speedup vs baseline: 61.6174x; 1.0030x over previous
"""Trainium2 Bass kernel for nn_CrossAttention (8-core data-parallel over batch).

Reference math (per batch b, chunk c):
  en = LayerNorm(e) ; q = en@Wq+bq ; k = h@Wk+bk ; v = h@Wv+bv
  attn = softmax(q@k^T / 8) ; o = attn@v ; out = o@Wo + bo + e

Host-side folding:  q = ((e-mu)*rstd) @ (ln_g[:,None]*Wq/8) + (ln_b@Wq+bq)/8
so the on-chip LN is just (e-mu)*rstd.

On-chip dataflow (all matmuls fp32r):
  - activations transposed to d-major ("T") layout via PE transposes
  - qT projection with weights as stationary lhsT; biases added by K=1
    piggyback matmuls (ones-row x bias-row) inside each accumulation group
  - kT and v stored in block-diagonal head-pair layout so scores and AV are
    single full-array K=128 matmuls per pair (fp32r cannot write PSUM at a
    column offset, which rules out tile_position pairing)
  - softmax denominators via one accumulated E2 matmul -> psum [12, 256];
    1/den partition-broadcast via K=12 matmul with R_p selector constants;
    normalization applied at AV-psum evacuation (oT = pav * bcast)
  - O-proj, PE transpose back to row-major with residual fused into the
    evacuation, store from the e tile (never fp32r-consumed)
  - two-stage software pipeline: stage A (load/LN/xT/Q/vbd) of chunk c is
    emitted before stage B (attention/O/store) of chunk c-1
"""

import numpy as np

B, C, N, S, D = 8, 32, 4, 64, 768
NH, DK = 12, 64
R = N * S          # 256 rows per chunk
KO = D // 128      # 6 partition blocks of d
NP = 6             # head pairs
LN_EPS = 1e-5
GROUP = 4          # chunks per h/kv batch group

_prog_cache = {}


def _build(n_chunks):
    import concourse.bass as bass
    import concourse.tile as tile
    from concourse import bacc, mybir
    from contextlib import ExitStack

    F32 = mybir.dt.float32
    F32R = mybir.dt.float32r
    AF = mybir.ActivationFunctionType
    ALU = mybir.AluOpType

    nc = bacc.Bacc()

    d_e = nc.dram_tensor("e", [n_chunks, R, D], F32, kind="ExternalInput")
    d_h = nc.dram_tensor("h", [n_chunks, S, D], F32R, kind="ExternalInput")
    d_wq = nc.dram_tensor("wq", [KO, 128, D], F32R, kind="ExternalInput")
    d_wk = nc.dram_tensor("wk", [KO, 128, D], F32R, kind="ExternalInput")
    d_wv = nc.dram_tensor("wv", [KO, 128, D], F32R, kind="ExternalInput")
    d_wo = nc.dram_tensor("wo", [KO, 128, D], F32R, kind="ExternalInput")
    d_bqc = nc.dram_tensor("bqc", [128, KO], F32, kind="ExternalInput")
    d_bkc = nc.dram_tensor("bkc", [128, KO], F32, kind="ExternalInput")
    d_boc = nc.dram_tensor("boc", [128, KO], F32, kind="ExternalInput")
    d_e2c = nc.dram_tensor("e2c", [128, NP, NH], F32R, kind="ExternalInput")
    d_rpc = nc.dram_tensor("rpc", [NH, NP, 128], F32R, kind="ExternalInput")
    d_id = nc.dram_tensor("ident", [128, 128], F32R, kind="ExternalInput")
    d_zf = nc.dram_tensor("zf", [128, 1], F32R, kind="ExternalInput")
    d_ones = nc.dram_tensor("ones", [1, 512], F32R, kind="ExternalInput")
    d_bvrr = nc.dram_tensor("bvrr", [1, D], F32R, kind="ExternalInput")
    d_out = nc.dram_tensor("out", [n_chunks, R, D], F32, kind="ExternalOutput")

    with ExitStack() as ctx:
        tc = ctx.enter_context(tile.TileContext(nc))
        consts = ctx.enter_context(tc.tile_pool(name="consts", bufs=1))
        e_pool = ctx.enter_context(tc.tile_pool(name="e_pool", bufs=2))
        x_pool = ctx.enter_context(tc.tile_pool(name="x_pool", bufs=2))
        xT_pool = ctx.enter_context(tc.tile_pool(name="xT_pool", bufs=2))
        q_pool = ctx.enter_context(tc.tile_pool(name="q_pool", bufs=2))
        exp_pool = ctx.enter_context(tc.tile_pool(name="exp_pool", bufs=2))
        oT_pool = ctx.enter_context(tc.tile_pool(name="oT_pool", bufs=1))
        fT_pool = ctx.enter_context(tc.tile_pool(name="fT_pool", bufs=1))
        vd_pool = ctx.enter_context(tc.tile_pool(name="vd_pool", bufs=2))
        grp_pool = ctx.enter_context(tc.tile_pool(name="grp_pool", bufs=1))
        kt_pool = ctx.enter_context(tc.tile_pool(name="kt_pool", bufs=1))
        v2_pool = ctx.enter_context(tc.tile_pool(name="v2_pool", bufs=2))
        st_pool = ctx.enter_context(tc.tile_pool(name="st_pool", bufs=2))
        ps_mmA = ctx.enter_context(tc.tile_pool(name="ps_mmA", bufs=2, space="PSUM"))
        ps_mmB = ctx.enter_context(tc.tile_pool(name="ps_mmB", bufs=2, space="PSUM"))
        ps_s = ctx.enter_context(tc.tile_pool(name="ps_s", bufs=1, space="PSUM"))
        ps_ab = ctx.enter_context(tc.tile_pool(name="ps_ab", bufs=3, space="PSUM"))

        # ---- constants ----
        wq = consts.tile([128, KO, D], F32R)
        wk = consts.tile([128, KO, D], F32R)
        wv = consts.tile([128, KO, D], F32R)
        wo = consts.tile([128, KO, D], F32R)
        nc.sync.dma_start(wq[:], d_wq[:].rearrange("k p d -> p k d"))
        nc.sync.dma_start(wk[:], d_wk[:].rearrange("k p d -> p k d"))
        nc.sync.dma_start(wv[:], d_wv[:].rearrange("k p d -> p k d"))
        nc.sync.dma_start(wo[:], d_wo[:].rearrange("k p d -> p k d"))
        bqc = consts.tile([128, KO], F32)
        bkc = consts.tile([128, KO], F32)
        boc = consts.tile([128, KO], F32)
        nc.sync.dma_start(bqc[:], d_bqc[:])
        nc.sync.dma_start(bkc[:], d_bkc[:])
        nc.sync.dma_start(boc[:], d_boc[:])
        e2c = consts.tile([128, NP, NH], F32R)
        rpc = consts.tile([NH, NP, 128], F32R)
        ident = consts.tile([128, 128], F32R)
        zf = consts.tile([128, 1], F32R)
        ones = consts.tile([1, 512], F32R)
        bvrr = consts.tile([1, D], F32R)
        nc.sync.dma_start(e2c[:], d_e2c[:])
        nc.sync.dma_start(rpc[:], d_rpc[:])
        nc.sync.dma_start(ident[:], d_id[:])
        nc.sync.dma_start(zf[:], d_zf[:])
        nc.sync.dma_start(ones[:], d_ones[:])
        nc.sync.dma_start(bvrr[:], d_bvrr[:])
        eps_t = consts.tile([128, 1], F32)
        nc.vector.memset(eps_t[:], LN_EPS)

        def group_phase(g):
            # h load (halves), transpose, K/V projections
            hT4 = grp_pool.tile([128, KO, GROUP * S], F32R, tag="hT4")
            for hh in range(2):
                h2 = grp_pool.tile([S, 2, D], F32R, tag="h2")
                nc.sync.dma_start(
                    h2[:], d_h[g * GROUP + 2 * hh:g * GROUP + 2 * hh + 2]
                    .rearrange("c j d -> j c d"))
                for c2 in range(2):
                    cc = 2 * hh + c2
                    for k0, kn in ((0, 4), (4, 2)):
                        ptq = ps_mmA.tile([128, 4, 128], F32R, tag="mmA")
                        for i in range(kn):
                            nc.tensor.transpose(
                                ptq[:, i, 0:S],
                                h2[:, c2, (k0 + i) * 128:(k0 + i + 1) * 128],
                                ident[0:S, 0:S])
                        nc.vector.tensor_copy(
                            hT4[:, k0:k0 + kn, cc * S:(cc + 1) * S],
                            ptq[:, 0:kn, 0:S])

            # kT in block-diagonal pair layout
            kbd = kt_pool.tile([128, NP, GROUP, 128], F32R, tag="kbd")
            nc.gpsimd.tensor_copy(
                kbd[:], zf[:, None, None, 0:1].to_broadcast(
                    [128, NP, GROUP, 128]))
            for mo in range(KO):
                pk = ps_mmA.tile([128, 512], F32, tag="mmA")
                for ko in range(KO):
                    nc.tensor.matmul(
                        pk[:, 0:GROUP * S],
                        wk[:, ko, mo * 128:(mo + 1) * 128],
                        hT4[:, ko, :],
                        start=(ko == 0), stop=(ko == KO - 1))
                pkv = pk[:, 0:GROUP * S].rearrange("p (c j) -> p c j", c=GROUP)
                nc.vector.tensor_scalar(
                    out=kbd[0:64, mo, :, 0:S], in0=pkv[0:64],
                    scalar1=bkc[0:64, mo:mo + 1], scalar2=None, op0=ALU.add)
                nc.vector.tensor_scalar(
                    out=kbd[64:128, mo, :, S:128], in0=pkv[64:128],
                    scalar1=bkc[64:128, mo:mo + 1], scalar2=None, op0=ALU.add)

            v2 = []
            for st in range(GROUP // 2):
                v2t = v2_pool.tile([128, D], F32R, tag="v2")
                for n0, ns in ((0, 512), (512, 256)):
                    pv = ps_mmA.tile([128, 512], F32, tag="mmA")
                    for ko in range(KO):
                        nc.tensor.matmul(
                            pv[:, 0:ns],
                            hT4[:, ko, st * 128:(st + 1) * 128],
                            wv[:, ko, n0:n0 + ns],
                            start=(ko == 0), stop=False)
                    nc.tensor.matmul(
                        pv[:, 0:ns], ones[:, 0:128],
                        bvrr[:, n0:n0 + ns], start=False, stop=True)
                    nc.vector.tensor_copy(v2t[:, n0:n0 + ns], pv[:, 0:ns])
                v2.append(v2t)
            return kbd, v2

        def stage_a(c, hctx):
            kbd, v2 = hctx
            cc = c % GROUP
            # ---- load e, LayerNorm stats + apply ----
            e_sb = e_pool.tile([128, 2, D], F32, tag="e")
            nc.sync.dma_start(
                e_sb[:], d_e[c].rearrange("(t p) d -> p t d", p=128))

            stats = st_pool.tile([128, 2, 3, 6], F32, tag="stats")
            mv = st_pool.tile([128, 2, 2], F32, tag="mv")
            rstd = st_pool.tile([128, 2], F32, tag="rstd")
            x_sb = x_pool.tile([128, 2, D], F32R, tag="x")
            for t in range(2):
                esl = e_sb[:, t, :].rearrange("p (s f) -> p s f", s=3)
                for sg in range(3):
                    nc.vector.bn_stats(stats[:, t, sg, :], esl[:, sg, :])
                nc.vector.bn_aggr(mv[:, t, :], stats[:, t, :, :])
            # rstd = rsqrt(var + eps) via bit-hack + 2 Newton steps (DVE only,
            # keeps Sqrt out of ACT so no act-table reloads)
            I32 = mybir.dt.int32
            v1 = st_pool.tile([128, 2], F32, tag="v1")
            y = st_pool.tile([128, 2], F32, tag="y")
            tmp = st_pool.tile([128, 2], F32, tag="tmp")
            nc.vector.tensor_scalar(
                out=v1[:], in0=mv[:, :, 1], scalar1=float(LN_EPS), scalar2=None,
                op0=ALU.add)
            nc.vector.tensor_scalar(
                out=y[:].bitcast(I32), in0=v1[:].bitcast(I32), scalar1=1,
                scalar2=None, op0=ALU.logical_shift_right)
            nc.vector.tensor_scalar(
                out=y[:].bitcast(I32), in0=y[:].bitcast(I32), scalar1=-1,
                scalar2=0x5F3759DF, op0=ALU.mult, op1=ALU.add)
            for _ in range(2):
                nc.vector.tensor_tensor(
                    out=tmp[:], in0=y[:], in1=y[:], op=ALU.mult)
                nc.vector.tensor_tensor(
                    out=tmp[:], in0=tmp[:], in1=v1[:], op=ALU.mult)
                nc.vector.tensor_scalar(
                    out=tmp[:], in0=tmp[:], scalar1=-0.5, scalar2=1.5,
                    op0=ALU.mult, op1=ALU.add)
                nc.vector.tensor_tensor(
                    out=rstd[:], in0=y[:], in1=tmp[:], op=ALU.mult)
                nc.vector.tensor_copy(y[:], rstd[:])
            for t in range(2):
                nc.gpsimd.tensor_scalar(
                    out=x_sb[:, t, :], in0=e_sb[:, t, :],
                    scalar1=mv[:, t, 0:1], scalar2=rstd[:, t:t + 1],
                    op0=ALU.subtract, op1=ALU.mult)

            # ---- transpose x to d-major ----
            xT = xT_pool.tile([128, KO, R], F32R, tag="xT")
            for ko2 in range(KO // 2):
                pt4 = ps_mmA.tile([128, 4, 128], F32R, tag="mmA")
                for i in range(2):
                    for t in range(2):
                        nc.tensor.transpose(
                            pt4[:, 2 * i + t, :],
                            x_sb[:, t, (2 * ko2 + i) * 128:(2 * ko2 + i + 1) * 128],
                            ident[:])
                nc.vector.tensor_copy(xT[:, 2 * ko2:2 * ko2 + 2, :], pt4[:])

            # ---- Q projection ----
            qT = q_pool.tile([128, KO, R], F32R, tag="qT")
            for mo in range(KO):
                pq = ps_mmA.tile([128, 512], F32, tag="mmA")
                for ko in range(KO):
                    nc.tensor.matmul(
                        pq[:, 0:R], wq[:, ko, mo * 128:(mo + 1) * 128],
                        xT[:, ko, :], start=(ko == 0), stop=(ko == KO - 1))
                nc.scalar.activation(
                    qT[:, mo, :], pq[:, 0:R], AF.Identity,
                    bias=bqc[:, mo:mo + 1], scale=1.0)

            # ---- v in block-diagonal pair layout ----
            v2t = v2[cc // 2]
            pa = 64 * (cc % 2)
            vbd = vd_pool.tile([128, NP, 128], F32R, tag="vbd")
            nc.gpsimd.tensor_copy(
                vbd[:], zf[:, None, 0:1].to_broadcast([128, NP, 128]))
            v2v = v2t[pa:pa + 64, :].rearrange(
                "p (np two dk) -> p np two dk", np=NP, two=2)
            nc.gpsimd.tensor_copy(vbd[0:64, :, 0:DK], v2v[:, :, 0, :])
            nc.gpsimd.tensor_copy(vbd[64:128, :, DK:128], v2v[:, :, 1, :])
            return (c, e_sb, x_sb, xT, qT, vbd)

        def stage_b(actx, hctx):
            c, e_sb, x_sb, xT, qT, vbd = actx
            kbd, v2 = hctx
            cc = c % GROUP

            # ---- attention (head-pairs processed two at a time) ----
            expT = exp_pool.tile([128, NP, R], F32R, tag="expT")
            pden_t = ps_s.tile([128, 2, R], F32, tag="s", name="pden_t")
            pden = pden_t[0:NH, 0, :]
            for p2 in range(0, NP, 2):
                pscr = ps_s.tile([128, 2, R], F32, tag="s")
                for i in range(2):
                    nc.tensor.matmul(
                        pscr[:, i, :], kbd[:, p2 + i, cc, :], qT[:, p2 + i, :],
                        start=True, stop=True)
                nc.scalar.activation(
                    expT[:, p2:p2 + 2, :], pscr[:], AF.Exp, bias=0.0, scale=1.0)
            for p in range(NP):
                nc.tensor.matmul(
                    pden[:], e2c[:, p, :], expT[:, p, :],
                    start=(p == 0), stop=(p == NP - 1),
                    skip_group_check=True)

            recip = st_pool.tile([NH, R], F32R, tag="recip")
            with nc.allow_low_precision(reason="fp32r softmax denom"):
                nc.vector.reciprocal(recip[:], pden[:])

            oT = oT_pool.tile([128, KO, R], F32R, tag="oT")
            for p2 in range(0, NP, 2):
                pav = ps_ab.tile([128, 2, R], F32, tag="ab")
                pbc = ps_ab.tile([128, 2, R], F32, tag="ab")
                for i in range(2):
                    nc.tensor.matmul(
                        pav[:, i, :], vbd[:, p2 + i, :], expT[:, p2 + i, :],
                        start=True, stop=True)
                    nc.tensor.matmul(
                        pbc[:, i, :], rpc[:, p2 + i, :], recip[:],
                        start=True, stop=True)
                bc_sb = st_pool.tile([128, 2, R], F32, tag="bcsb")
                nc.scalar.copy(bc_sb[:], pbc[:])
                nc.vector.tensor_tensor(
                    out=oT[:, p2:p2 + 2, :], in0=pav[:], in1=bc_sb[:],
                    op=ALU.mult)

            # ---- O projection ----
            fT = fT_pool.tile([128, KO, R], F32R, tag="fT")
            for mo in range(KO):
                pf = ps_mmB.tile([128, 512], F32, tag="mmB")
                for ko in range(KO):
                    nc.tensor.matmul(
                        pf[:, 0:R], wo[:, ko, mo * 128:(mo + 1) * 128],
                        oT[:, ko, :], start=(ko == 0), stop=(ko == KO - 1))
                nc.scalar.activation(
                    fT[:, mo, :], pf[:, 0:R], AF.Identity,
                    bias=boc[:, mo:mo + 1], scale=1.0)

            # ---- transpose back + residual, store ----
            for t in range(2):
                for m0, mn in ((0, 4), (4, 2)):
                    ptq = ps_mmB.tile([128, 4, 128], F32R, tag="mmB")
                    for i in range(mn):
                        nc.tensor.transpose(
                            ptq[:, i, :], fT[:, m0 + i, t * 128:(t + 1) * 128],
                            ident[:])
                    nc.vector.tensor_tensor(
                        out=e_sb[:, t, m0 * 128:(m0 + mn) * 128],
                        in0=ptq[:, 0:mn, :].bitcast(F32),
                        in1=e_sb[:, t, m0 * 128:(m0 + mn) * 128],
                        op=ALU.add)
            nc.sync.dma_start(
                d_out[c].rearrange("(t p) d -> p t d", p=128), e_sb[:])

        # ---- software-pipelined driver: A(c+1) emitted ahead of B(c) ----
        n_groups = n_chunks // GROUP
        pending = None
        for g in range(n_groups):
            hctx = group_phase(g)
            for cc in range(GROUP):
                actx = stage_a(g * GROUP + cc, hctx)
                if pending is not None:
                    stage_b(*pending)
                pending = (actx, hctx)
        if pending is not None:
            stage_b(*pending)

    nc.compile()
    return nc


def _prep_consts(Wq, bq, Wk, bk, Wv, bv, Wo, bo, ln_g, ln_b):
    scale = 1.0 / np.sqrt(DK)
    Wq_eff = (ln_g[:, None] * Wq) * scale
    bq_eff = (ln_b @ Wq + bq) * scale

    def wl(w):
        return np.ascontiguousarray(w.reshape(KO, 128, D), dtype=np.float32)

    e2c = np.zeros((128, NP, NH), dtype=np.float32)
    for p in range(NP):
        e2c[0:64, p, 2 * p] = 1.0
        e2c[64:128, p, 2 * p + 1] = 1.0
    rpc = np.zeros((NH, NP, 128), dtype=np.float32)
    for p in range(NP):
        rpc[2 * p, p, 0:64] = 1.0
        rpc[2 * p + 1, p, 64:128] = 1.0

    return {
        "wq": wl(Wq_eff), "wk": wl(Wk), "wv": wl(Wv), "wo": wl(Wo),
        "e2c": e2c, "rpc": rpc, "zf": np.zeros((128, 1), dtype=np.float32),
        "ones": np.ones((1, 512), dtype=np.float32),
        "bqc": np.ascontiguousarray(bq_eff.reshape(KO, 128).T, dtype=np.float32),
        "bkc": np.ascontiguousarray(bk.reshape(KO, 128).T, dtype=np.float32),
        "boc": np.ascontiguousarray(bo.reshape(KO, 128).T, dtype=np.float32),
        "bvrr": np.ascontiguousarray(bv.reshape(1, D), dtype=np.float32),
        "ident": np.eye(128, dtype=np.float32),
    }


def kernel(e, h, Wq, bq, Wk, bk, Wv, bv, Wo, bo, ln_g, ln_b):
    from concourse.bass_utils import run_bass_kernel_spmd

    e = np.asarray(e, dtype=np.float32)
    h = np.asarray(h, dtype=np.float32)
    n_chunks = e.shape[1]

    if n_chunks not in _prog_cache:
        _prog_cache[n_chunks] = _build(n_chunks)
    nc = _prog_cache[n_chunks]

    consts = _prep_consts(
        np.asarray(Wq, np.float32), np.asarray(bq, np.float32),
        np.asarray(Wk, np.float32), np.asarray(bk, np.float32),
        np.asarray(Wv, np.float32), np.asarray(bv, np.float32),
        np.asarray(Wo, np.float32), np.asarray(bo, np.float32),
        np.asarray(ln_g, np.float32), np.asarray(ln_b, np.float32))

    in_maps = []
    for b in range(B):
        m = dict(consts)
        m["e"] = np.ascontiguousarray(e[b].reshape(n_chunks, R, D))
        m["h"] = np.ascontiguousarray(h[b])
        in_maps.append(m)

    res = run_bass_kernel_spmd(nc, in_maps, core_ids=list(range(B)))
    out = np.stack([r["out"] for r in res.results], axis=0)
    return out.reshape(B, n_chunks, N, S, D)
